# revision 13
# baseline (speedup 1.0000x reference)
"""Trainium2 Bass kernel for nn_ExpertsChooseMaskedExpand (MoE routing).

Reference computes (per batch b):
    xd[e,c,j] = sum_t mask[t,e,c] * x[t,e,j]          (dispatch)
    y[e,c,o]  = sum_j xd[e,c,j] * w[e,o,j] + bias[o]  (expert GEMM)
    out[t,o]  = sum_{e,c} comb[t,e,c] * y[e,c,o]      (combine)

We use associativity to contract comb with xd first:
    z[t,e,j] = sum_c comb[t,e,c] * xd[e,c,j]
    out[t,o] = sum_{e,j} z[t,e,j] * w[e,o,j] + bias[o] * S[t],
    S[t] = sum_{e,c} comb[t,e,c]
which cuts FLOPs ~3.4x and never materializes y (B,E,C,O).

Sharding: 8 cores; core k handles batch b=k//2 and expert group
h=k%2 (experts h*4..h*4+4) over ALL 4096 tokens. Each core produces a
partial out (T, O) summed over its 4 experts only; the host adds the
two partials of each batch pair (plus bias*S).

Precision: mask and comb stream as fp8 e3m4. Their quantization noise
is zero-mean and incoherent while the signal rides a coherent mean-0.5
path through both contractions, so the final output error contribution
is only ~1.2e-3 (measured) vs the 2e-2 gate. x/w/z stay bf16 (the z@w
GEMM operands get no such attenuation). fp8 streams halve the head's
mask DMA and the tail's comb DMA.

Schedule: head = dispatch per expert paced by the fp8 mask stream
(sync ring), xh on scalar, cb(tch0)+wf on the gpsimd ring. Remaining
head PE idle is filled by staging tch0 combine chains over experts
0..1 into bf16 partials (pout); the tail finishes those tiles with a
2-matmul (e2,e3) chain merged via a DVE add at drain time (no PE
inject). Tail is PE-bound at the ~216ns/matmul N=512 issue floor.
"""

import numpy as np
import ml_dtypes

BF16 = ml_dtypes.bfloat16
F8E3 = ml_dtypes.float8_e3m4

B, T, E, C = 4, 4096, 8, 512
I = 128            # per-expert input features
O = 4096           # out_features
NCORES = 8
EL = E // 2        # experts per core
NTT = T // 128     # 32 token tiles
NCT = C // 128     # 4 c-tiles
NTCH = T // 512    # 8 t-chunks (z / combine granularity)
NOT2 = O // 1024   # 4 o-slices of the weight DRAM layout

STAGE_TCHS = (0,)  # t-chunks whose (e0,e1) combine partials are staged in head

_CACHE = {}


def _build():
    import concourse.bass as bass
    import concourse.tile as tile
    import concourse.bacc as bacc
    import concourse.mybir as mybir

    f32 = mybir.dt.float32
    bf16 = mybir.dt.bfloat16
    f8 = mybir.dt.float8e3
    ts = bass.ts

    nc = bacc.Bacc(None, target_bir_lowering=False, debug=False)

    xh = nc.dram_tensor("xh", [EL, 128, NTT, I], bf16, kind="ExternalInput")
    mh = nc.dram_tensor("mh", [EL, 128, NTT, C], f8, kind="ExternalInput")
    # comb packed per (expert, t-chunk): contiguous 2KB/partition bursts
    cbp = nc.dram_tensor("cbp", [EL, NTCH, 128, NCT * 512], f8,
                         kind="ExternalInput")
    wf = nc.dram_tensor("wf", [NOT2, 128, EL, 1024], bf16,
                        kind="ExternalInput")
    ident = nc.dram_tensor("ident", [128, 128], bf16, kind="ExternalInput")
    out_d = nc.dram_tensor("out", [T, O], bf16, kind="ExternalOutput")

    with tile.TileContext(nc) as tc:
        with (
            tc.tile_pool(name="persist", bufs=1) as persist,
            tc.tile_pool(name="psumB", bufs=1, space="PSUM") as psumb,
        ):
            wf_sb = persist.tile([128, EL, O], bf16, tag="wf")
            id_sb = persist.tile([128, 128], bf16, tag="ident")
            nc.gpsimd.dma_start(id_sb[:], ident[:])

            xd = {}    # e -> xd tile [128c, (ct j)] bf16
            zt = {}    # (e, tch) -> z^T tile [128j, 512t] bf16
            pout = {}  # (tch, tt, ot) -> staged (e0+e1) partial, bf16

            def cb_load(e, tch, cb_pool, cb_bufs, eng):
                cb_t = cb_pool.tile([128, NCT * 512], f8, tag="cb",
                                    bufs=cb_bufs, name=f"cb{e}_{tch}")
                eng.dma_start(cb_t[:], cbp[e, tch])
                return cb_t

            def stage_b(e, tch, cb_t):
                # z^T[e][tch][j, t] = sum_c xd[e][c, j] * comb^T[c, t]
                ps_b = psumb.tile([128, 512], f32, tag="psB", bufs=2,
                                  name=f"psB{e}_{tch}")
                for ct in range(NCT):
                    nc.tensor.matmul(
                        ps_b[:],
                        xd[e][:, ts(ct, 128)],
                        cb_t[:, ts(ct, 512)],
                        start=(ct == 0),
                        stop=(ct == NCT - 1),
                    )
                z_sb = persist.tile([128, 512], bf16, tag=f"zt{e}_{tch}",
                                    name=f"zt{e}_{tch}")
                nc.vector.tensor_copy(z_sb[:], ps_b[:])
                zt[(e, tch)] = z_sb

            # ---- Head phase: dispatch, paced by the fp8 mask stream ----
            # staged combine chains (e0+e1), emitted interleaved into the
            # e2/e3 dispatch chunk slots so they fill mask-wait PE idle
            stage_list = [(tch, tt, ot)
                          for ot in range(NOT2 * 2)
                          for tch in STAGE_TCHS
                          for tt in range(4)]
            n_slots = 2 * (NTT // 8)  # q-slots across e2+e3

            with (
                tc.tile_pool(name="head", bufs=1) as head,
                tc.tile_pool(name="psumD", bufs=1, space="PSUM") as psumd,
            ):
                def pout_chain(tch, tt, ot):
                    ps_p = psumd.tile([128, 512], f32, tag="psP", bufs=2,
                                      name=f"psP{tch}_{tt}_{ot}")
                    for pe in (0, 1):
                        nc.tensor.matmul(
                            ps_p[:],
                            zt[(pe, tch)][:, ts(tt, 128)],
                            wf_sb[:, pe, ts(ot, 512)],
                            start=(pe == 0), stop=(pe == 1),
                        )
                    po = persist.tile([128, 512], bf16,
                                      tag=f"po{tch}_{tt}_{ot}",
                                      name=f"po{tch}_{tt}_{ot}")
                    nc.vector.tensor_copy(po[:], ps_p[:])
                    pout[(tch, tt, ot)] = po

                # All head load triggers are emitted upfront, in ring order.
                # The mask is split across BOTH HWDGE rings (e0/e2 on sync,
                # e1/e3 on scalar) so consecutive experts' masks stream
                # concurrently, and triggers never sit behind compute ops.
                NQ = NTT // 8
                xh_tiles = {}
                mh_tiles = {}
                # sync ring: mask e0, then wf (needed by the staged chains
                # in e2's slots), then mask e2
                for e in (0, 2):
                    for q in range(NQ):
                        mh_t = head.tile([128, 8, C], f8, tag="mha", bufs=8,
                                         name=f"mh{e}_{q}")
                        nc.sync.dma_start(mh_t[:],
                                          mh[e, :, q * 8:q * 8 + 8, :])
                        mh_tiles[(e, q)] = mh_t
                    if e == 0:
                        for ot2 in range(NOT2):
                            nc.sync.dma_start(
                                wf_sb[:, :, ts(ot2, 1024)], wf[ot2])
                # scalar ring: xh0 (split), xh1, mask e1, xh2, xh3, mask e3
                for e in range(EL):
                    xh_t = head.tile([128, NTT, I], bf16, tag="xh", bufs=3,
                                     name=f"xh{e}")
                    if e == 0:
                        nc.scalar.dma_start(xh_t[:, 0:16, :],
                                            xh[e, :, 0:16, :])
                        nc.scalar.dma_start(xh_t[:, 16:32, :],
                                            xh[e, :, 16:32, :])
                    else:
                        nc.scalar.dma_start(xh_t[:], xh[e])
                    xh_tiles[e] = xh_t
                    if e in (1, 3):
                        for q in range(NQ):
                            mh_t = head.tile([128, 8, C], f8, tag="mhb",
                                             bufs=8, name=f"mh{e}_{q}")
                            nc.scalar.dma_start(
                                mh_t[:], mh[e, :, q * 8:q * 8 + 8, :])
                            mh_tiles[(e, q)] = mh_t
                # gpsimd ring: cb for the staged t-chunks of every expert
                cb_head = {(e, tch): cb_load(e, tch, head, 4, nc.gpsimd)
                           for e in range(EL) for tch in STAGE_TCHS}

                for e in range(EL):
                    xh_t = xh_tiles[e]
                    # xd^T accumulator: [128j, 512c], one chain over all tt
                    ps_d = psumd.tile([128, C], f32, tag="psD", bufs=2,
                                      name=f"psD{e}")
                    for q in range(NQ):
                        mh_t = mh_tiles[(e, q)]
                        if e >= 2:
                            # staged chains emitted BEFORE the mask-paced
                            # dispatch MMs of this slot: they are ready (wf
                            # rides ahead of e2/e3 mask on the sync ring) and
                            # fill the PE idle while the mask chunk arrives
                            slot = (e - 2) * (NTT // 8) + q
                            k0 = len(stage_list) * slot // n_slots
                            k1 = len(stage_list) * (slot + 1) // n_slots
                            for (tch, tt_, ot) in stage_list[k0:k1]:
                                pout_chain(tch, tt_, ot)
                        for i in range(8):
                            tt = q * 8 + i
                            nc.tensor.matmul(
                                ps_d[:],
                                xh_t[:, tt, :],
                                mh_t[:, i, :],
                                start=(tt == 0),
                                stop=(tt == NTT - 1),
                            )
                    xdT_sb = head.tile([128, C], bf16, tag="xdT", bufs=2,
                                       name=f"xdT{e}")
                    xd_sb = persist.tile([128, C], bf16, tag=f"xd{e}",
                                         name=f"xd{e}")
                    for ct in range(NCT):
                        # scalar engine: keeps the cast off the vector queue
                        nc.scalar.copy(xdT_sb[:, ts(ct, 128)],
                                       ps_d[:, ts(ct, 128)])
                        ps_t = psumd.tile([128, 128], bf16, tag="psT", bufs=2,
                                          name=f"psT{e}_{ct}")
                        nc.tensor.transpose(ps_t[:], xdT_sb[:, ts(ct, 128)],
                                            id_sb[:])
                        nc.vector.tensor_copy(xd_sb[:, ts(ct, 128)], ps_t[:])
                    xd[e] = xd_sb
                    for tch in STAGE_TCHS:
                        stage_b(e, tch, cb_head[(e, tch)])

            # ---- Combine phase (PE-bound), stage B interleaved per tch ----
            with (
                tc.tile_pool(name="tail", bufs=1) as tail,
                tc.tile_pool(name="psumC", bufs=1, space="PSUM") as psumc,
            ):
                for tcg in range(NTCH):
                    staged = tcg in STAGE_TCHS
                    final = tcg == NTCH - 1
                    for tt in range(tcg * 4, tcg * 4 + 4):
                        m = tt % 4
                        if not final:
                            out_sb = tail.tile([128, O], bf16, tag="out",
                                               bufs=4, name=f"out{tt}")
                        for ot in range(NOT2 * 2):
                            if final and ot % 2 == 0:
                                # final t-chunk: small pair tiles, written
                                # out immediately -- no 1MB serial tail and
                                # no buffer-aliasing stall on the last tiles
                                out_sb = tail.tile([128, 1024], bf16,
                                                   tag="out7", bufs=8,
                                                   name=f"out7_{tt}_{ot}")
                            osl = out_sb[:, ts(ot % 2 if final else ot, 512)]
                            ps_c = psumc.tile([128, 512], f32, tag="psC",
                                              bufs=5, name=f"psC{tt}_{ot}")
                            es = (2, 3) if staged else (0, 1, 2, 3)
                            for k, e in enumerate(es):
                                nc.tensor.matmul(
                                    ps_c[:],
                                    zt[(e, tcg)][:, ts(m, 128)],
                                    wf_sb[:, e, ts(ot, 512)],
                                    start=(k == 0),
                                    stop=(k == len(es) - 1),
                                )
                            if staged:
                                # merge the staged (e0+e1) partial at drain
                                nc.vector.tensor_tensor(
                                    osl, ps_c[:], pout[(tcg, m, ot)][:],
                                    mybir.AluOpType.add)
                            elif ot % 2 == 0:
                                nc.vector.tensor_copy(osl, ps_c[:])
                            else:
                                nc.scalar.copy(osl, ps_c[:])
                            if final and ot % 2 == 1:
                                nc.sync.dma_start(
                                    out_d[ts(tt, 128),
                                          (ot - 1) * 512:(ot + 1) * 512],
                                    out_sb[:])
                        if not final:
                            nc.sync.dma_start(out_d[ts(tt, 128), :],
                                              out_sb[:])
                            # one z chain per token tile keeps the psB /
                            # vector load smooth
                            e = tt % 4
                            stage_b(e, tcg + 1,
                                    cb_load(e, tcg + 1, tail, 6, nc.gpsimd))

    nc.compile()
    return nc


def _prep_inputs(x, weight, bias, combine_array, dispatch_mask):
    """Host-side cast + re-layout for contiguous device DMA."""
    x = np.asarray(x, np.float32)
    weight = np.asarray(weight, np.float32)
    bias = np.asarray(bias, np.float32)
    comb = np.asarray(combine_array, np.float32)
    mask = np.asarray(dispatch_mask, np.float32)

    # xh: (B, E, 128, NTT, I); xh[b, e, p, tt, j] = x[b, tt*128+p, e, j]
    xh = np.ascontiguousarray(
        x.reshape(B, NTT, 128, E, I).transpose(0, 3, 2, 1, 4)).astype(BF16)
    # mh: (B, E, 128, NTT, C) in fp8 e3m4
    mh = np.ascontiguousarray(
        mask.reshape(B, NTT, 128, E, C).transpose(0, 3, 2, 1, 4)).astype(F8E3)
    # cbp: (B, E, NTCH, 128, NCT*512) fp8;
    # cbp[b, e, tch, pc, ct*512+tq] = comb[b, tch*512+tq, e, ct*128+pc]
    cbp = np.ascontiguousarray(
        comb.reshape(B, NTCH, 512, E, NCT, 128).transpose(0, 3, 1, 5, 4, 2)
    ).astype(F8E3).reshape(B, E, NTCH, 128, NCT * 512)
    # wf: (NOT2, 128, E, 1024); wf[ot, j, e, oq] =
    #     weight.reshape(E, O, I)[e, ot*1024+oq, j]
    wfa = np.ascontiguousarray(
        weight.reshape(E, NOT2, 1024, I).transpose(1, 3, 0, 2)).astype(BF16)
    # S[b, t] = sum_{e,c} comb[b, t, e, c] -- bias*S added on host in f32
    s = comb.sum(axis=(2, 3))
    idm = np.eye(128, dtype=BF16)

    in_maps = []
    for k in range(NCORES):
        b, h = k // 2, k % 2
        es = slice(h * EL, (h + 1) * EL)
        in_maps.append({
            "xh": np.ascontiguousarray(xh[b, es]),
            "mh": np.ascontiguousarray(mh[b, es]),
            "cbp": np.ascontiguousarray(cbp[b, es]),
            "wf": np.ascontiguousarray(wfa[:, :, es, :]),
            "ident": idm,
        })
    return in_maps, s, bias


def kernel(x, weight, bias, combine_array, dispatch_mask):
    from concourse import bass_utils

    if "nc" not in _CACHE:
        _CACHE["nc"] = _build()
    nc = _CACHE["nc"]

    in_maps, s, bias_f = _prep_inputs(
        x, weight, bias, combine_array, dispatch_mask)
    res = bass_utils.run_bass_kernel_spmd(
        nc, in_maps, core_ids=list(range(NCORES)))
    out = np.empty((B, T, O), np.float32)
    for b in range(B):
        out[b] = res.results[2 * b]["out"].astype(np.float32)
        out[b] += res.results[2 * b + 1]["out"].astype(np.float32)
    out += s[:, :, None] * bias_f[None, None, :]
    return out


# revision 15
# speedup vs baseline: 1.0638x; 1.0638x over previous
"""Trainium2 Bass kernel for nn_ExpertsChooseMaskedExpand (MoE routing).

Reference computes (per batch b):
    xd[e,c,j] = sum_t mask[t,e,c] * x[t,e,j]          (dispatch)
    y[e,c,o]  = sum_j xd[e,c,j] * w[e,o,j] + bias[o]  (expert GEMM)
    out[t,o]  = sum_{e,c} comb[t,e,c] * y[e,c,o]      (combine)

We use associativity to contract comb with xd first:
    z[t,e,j] = sum_c comb[t,e,c] * xd[e,c,j]
    out[t,o] = sum_{e,j} z[t,e,j] * w[e,o,j] + bias[o] * S[t],
    S[t] = sum_{e,c} comb[t,e,c]
which cuts FLOPs ~3.4x and never materializes y (B,E,C,O).

Sharding: 8 cores; core k handles batch b=k//2 and expert group
h=k%2 (experts h*4..h*4+4) over ALL 4096 tokens. Each core produces a
partial out (T, O) summed over its 4 experts only; the host adds the
two partials of each batch pair (plus bias*S).

Precision: mask and comb stream as fp8 e3m4. Their quantization noise
is zero-mean and incoherent while the signal rides a coherent mean-0.5
path through both contractions, so the final output error contribution
is only ~1.2e-3 (measured) vs the 2e-2 gate. x/w/z stay bf16 (the z@w
GEMM operands get no such attenuation). fp8 streams halve the head's
mask DMA and the tail's comb DMA.

Schedule: head = dispatch per expert paced by the fp8 mask stream
(sync ring), xh on scalar, cb(tch0)+wf on the gpsimd ring. Remaining
head PE idle is filled by staging tch0 combine chains over experts
0..1 into bf16 partials (pout); the tail finishes those tiles with a
2-matmul (e2,e3) chain merged via a DVE add at drain time (no PE
inject). Tail is PE-bound at the ~216ns/matmul N=512 issue floor.
"""

import numpy as np
import ml_dtypes

BF16 = ml_dtypes.bfloat16
F8E3 = ml_dtypes.float8_e3m4

B, T, E, C = 4, 4096, 8, 512
I = 128            # per-expert input features
O = 4096           # out_features
NCORES = 8
EL = E // 2        # experts per core
NTT = T // 128     # 32 token tiles
NCT = C // 128     # 4 c-tiles
NTCH = T // 512    # 8 t-chunks (z / combine granularity)
NOT2 = O // 1024   # 4 o-slices of the weight DRAM layout

STAGE_TCHS = (0,)  # t-chunks whose (e0,e1) combine partials are staged in head

_CACHE = {}


def _build():
    import concourse.bass as bass
    import concourse.tile as tile
    import concourse.bacc as bacc
    import concourse.mybir as mybir

    f32 = mybir.dt.float32
    bf16 = mybir.dt.bfloat16
    f8 = mybir.dt.float8e3
    ts = bass.ts

    nc = bacc.Bacc(None, target_bir_lowering=False, debug=False)

    xh = nc.dram_tensor("xh", [EL, 128, NTT, I], bf16, kind="ExternalInput")
    mh = nc.dram_tensor("mh", [EL, 128, NTT, C], f8, kind="ExternalInput")
    # comb packed per (expert, t-chunk): contiguous 2KB/partition bursts
    cbp = nc.dram_tensor("cbp", [EL, NTCH, 128, NCT * 512], f8,
                         kind="ExternalInput")
    wfp = nc.dram_tensor("wfp", [2, 128, EL * 2048], bf16,
                         kind="ExternalInput")
    ident = nc.dram_tensor("ident", [128, 128], bf16, kind="ExternalInput")
    out_d = nc.dram_tensor("out", [T, O], bf16, kind="ExternalOutput")

    with tile.TileContext(nc) as tc:
        with (
            tc.tile_pool(name="persist", bufs=1) as persist,
            tc.tile_pool(name="psumB", bufs=1, space="PSUM") as psumb,
        ):
            wf_sb = persist.tile([128, EL, O], bf16, tag="wf")
            id_sb = persist.tile([128, 128], bf16, tag="ident")
            nc.gpsimd.dma_start(id_sb[:], ident[:])

            xd = {}    # e -> xd tile [128c, (ct j)] bf16
            zt = {}    # (e, tch) -> z^T tile [128j, 512t] bf16
            pout = {}  # (tch, tt, ot) -> staged (e0+e1) partial, bf16

            def cb_load(e, tch, cb_pool, cb_bufs, eng):
                cb_t = cb_pool.tile([128, NCT * 512], f8, tag="cb",
                                    bufs=cb_bufs, name=f"cb{e}_{tch}")
                eng.dma_start(cb_t[:], cbp[e, tch])
                return cb_t

            def stage_b(e, tch, cb_t):
                # z^T[e][tch][j, t] = sum_c xd[e][c, j] * comb^T[c, t]
                ps_b = psumb.tile([128, 512], f32, tag="psB", bufs=2,
                                  name=f"psB{e}_{tch}")
                for ct in range(NCT):
                    nc.tensor.matmul(
                        ps_b[:],
                        xd[e][:, ts(ct, 128)],
                        cb_t[:, ts(ct, 512)],
                        start=(ct == 0),
                        stop=(ct == NCT - 1),
                    )
                z_sb = persist.tile([128, 512], bf16, tag="zt",
                                    bufs=EL * NTCH, name=f"zt{e}_{tch}")
                nc.vector.tensor_copy(z_sb[:], ps_b[:])
                zt[(e, tch)] = z_sb

            # ---- Head phase: dispatch, paced by the fp8 mask stream ----
            # staged combine chains (e0+e1), emitted interleaved into the
            # e2/e3 dispatch chunk slots so they fill mask-wait PE idle
            stage_list = [(tch, tt, ot)
                          for ot in range(NOT2 * 2)
                          for tch in STAGE_TCHS
                          for tt in range(4)]
            n_slots = 2 * (NTT // 8)  # q-slots across e2+e3

            with (
                tc.tile_pool(name="head", bufs=1) as head,
                tc.tile_pool(name="psumD", bufs=1, space="PSUM") as psumd,
            ):
                def pout_chain(tch, tt, ot):
                    ps_p = psumd.tile([128, 512], f32, tag="psP", bufs=2,
                                      name=f"psP{tch}_{tt}_{ot}")
                    for pe in (0, 1):
                        nc.tensor.matmul(
                            ps_p[:],
                            zt[(pe, tch)][:, ts(tt, 128)],
                            wf_sb[:, pe, ts(ot, 512)],
                            start=(pe == 0), stop=(pe == 1),
                        )
                    po = persist.tile([128, 512], bf16,
                                      tag="po", bufs=4 * len(STAGE_TCHS) * 8,
                                      name=f"po{tch}_{tt}_{ot}")
                    nc.vector.tensor_copy(po[:], ps_p[:])
                    pout[(tch, tt, ot)] = po

                # Ring plan. The SDMA engines round-robin between rings with
                # pending work, so bandwidth priority = keeping non-critical
                # transfers out of the early window. Sync ring carries ONLY
                # the mask (the stream that paces dispatch). The scalar ring
                # carries xh, then dep-gated xh3/wf (their triggers sit
                # behind e0's psD-dependent casts, so those transfers start
                # only once dispatch is underway). cb rides gpsimd.
                NQ = NTT // 8
                xh_tiles = {}
                mh_tiles = {}
                # sync ring: all masks, in expert order, nothing else
                for e in range(EL):
                    for q in range(NQ):
                        mh_t = head.tile([128, 8, C], f8, tag="mha", bufs=8,
                                         name=f"mh{e}_{q}")
                        nc.sync.dma_start(mh_t[:],
                                          mh[e, :, q * 8:q * 8 + 8, :])
                        mh_tiles[(e, q)] = mh_t
                # scalar ring: xh0 (split), xh1, xh2 upfront; xh3 + wf are
                # emitted after e0's casts below, so they are dep-gated
                for e in range(3):
                    xh_t = head.tile([128, NTT, I], bf16, tag="xh", bufs=3,
                                     name=f"xh{e}")
                    if e == 0:
                        nc.scalar.dma_start(xh_t[:, 0:16, :],
                                            xh[e, :, 0:16, :])
                        nc.scalar.dma_start(xh_t[:, 16:32, :],
                                            xh[e, :, 16:32, :])
                    else:
                        nc.scalar.dma_start(xh_t[:], xh[e])
                    xh_tiles[e] = xh_t
                # gpsimd ring: cb for the staged t-chunks of every expert
                cb_head = {(e, tch): cb_load(e, tch, head, 4, nc.gpsimd)
                           for e in range(EL) for tch in STAGE_TCHS}

                for e in range(EL):
                    xh_t = xh_tiles.get(e)
                    # xd^T accumulator: [128j, 512c], one chain over all tt
                    ps_d = psumd.tile([128, C], f32, tag="psD", bufs=2,
                                      name=f"psD{e}")
                    for q in range(NQ):
                        mh_t = mh_tiles[(e, q)]
                        if e >= 2:
                            # staged chains emitted BEFORE the mask-paced
                            # dispatch MMs of this slot: they are ready (wf
                            # rides ahead of e2/e3 mask on the sync ring) and
                            # fill the PE idle while the mask chunk arrives
                            slot = (e - 2) * (NTT // 8) + q
                            k0 = len(stage_list) * slot // n_slots
                            k1 = len(stage_list) * (slot + 1) // n_slots
                            for (tch, tt_, ot) in stage_list[k0:k1]:
                                pout_chain(tch, tt_, ot)
                        for i in range(8):
                            tt = q * 8 + i
                            nc.tensor.matmul(
                                ps_d[:],
                                xh_tiles[e][:, tt, :],
                                mh_t[:, i, :],
                                start=(tt == 0),
                                stop=(tt == NTT - 1),
                            )
                    xdT_sb = head.tile([128, C], bf16, tag="xdT", bufs=2,
                                       name=f"xdT{e}")
                    xd_sb = persist.tile([128, C], bf16, tag="xd", bufs=EL,
                                         name=f"xd{e}")
                    for ct in range(NCT):
                        # scalar engine: keeps the cast off the vector queue
                        nc.scalar.copy(xdT_sb[:, ts(ct, 128)],
                                       ps_d[:, ts(ct, 128)])
                        ps_t = psumd.tile([128, 128], bf16, tag="psT", bufs=2,
                                          name=f"psT{e}_{ct}")
                        nc.tensor.transpose(ps_t[:], xdT_sb[:, ts(ct, 128)],
                                            id_sb[:])
                        nc.vector.tensor_copy(xd_sb[:, ts(ct, 128)], ps_t[:])
                    if e == 0:
                        # dep-gated loads: these triggers sit on the scalar
                        # engine after e0's psD-dependent casts, so their
                        # transfers start only once dispatch is underway and
                        # never steal early bandwidth from the mask stream
                        xh_t3 = head.tile([128, NTT, I], bf16, tag="xh",
                                          bufs=3, name="xh3")
                        nc.scalar.dma_start(xh_t3[:], xh[3])
                        xh_tiles[3] = xh_t3
                        for k in range(2):
                            nc.scalar.dma_start(
                                wf_sb[:, :, ts(k, 2048)], wfp[k])
                    xd[e] = xd_sb
                    for tch in STAGE_TCHS:
                        stage_b(e, tch, cb_head[(e, tch)])

            # ---- Combine phase (PE-bound), stage B interleaved per tch ----
            with (
                tc.tile_pool(name="tail", bufs=1) as tail,
                tc.tile_pool(name="psumC", bufs=1, space="PSUM") as psumc,
            ):
                for tcg in range(NTCH):
                    staged = tcg in STAGE_TCHS
                    final = tcg == NTCH - 1
                    for tt in range(tcg * 4, tcg * 4 + 4):
                        m = tt % 4
                        if not final:
                            out_sb = tail.tile([128, O], bf16, tag="out",
                                               bufs=3, name=f"out{tt}")
                        for ot in range(NOT2 * 2):
                            if final and ot % 2 == 0:
                                # final t-chunk: small pair tiles, written
                                # out immediately -- no 1MB serial tail and
                                # no buffer-aliasing stall on the last tiles
                                out_sb = tail.tile([128, 1024], bf16,
                                                   tag="out7", bufs=16,
                                                   name=f"out7_{tt}_{ot}")
                            osl = out_sb[:, ts(ot % 2 if final else ot, 512)]
                            ps_c = psumc.tile([128, 512], f32, tag="psC",
                                              bufs=5, name=f"psC{tt}_{ot}")
                            es = (2, 3) if staged else (0, 1, 2, 3)
                            for k, e in enumerate(es):
                                nc.tensor.matmul(
                                    ps_c[:],
                                    zt[(e, tcg)][:, ts(m, 128)],
                                    wf_sb[:, e, ts(ot, 512)],
                                    start=(k == 0),
                                    stop=(k == len(es) - 1),
                                )
                            if staged:
                                # merge the staged (e0+e1) partial at drain
                                nc.vector.tensor_tensor(
                                    osl, ps_c[:], pout[(tcg, m, ot)][:],
                                    mybir.AluOpType.add)
                            elif ot % 2 == 0:
                                nc.vector.tensor_copy(osl, ps_c[:])
                            else:
                                nc.scalar.copy(osl, ps_c[:])
                            if final and ot % 2 == 1:
                                nc.sync.dma_start(
                                    out_d[ts(tt, 128),
                                          (ot - 1) * 512:(ot + 1) * 512],
                                    out_sb[:])
                        if not final:
                            nc.sync.dma_start(out_d[ts(tt, 128), :],
                                              out_sb[:])
                            # one z chain per token tile keeps the psB /
                            # vector load smooth
                            e = tt % 4
                            stage_b(e, tcg + 1,
                                    cb_load(e, tcg + 1, tail, 6, nc.gpsimd))

    nc.compile()
    return nc


def _prep_inputs(x, weight, bias, combine_array, dispatch_mask):
    """Host-side cast + re-layout for contiguous device DMA."""
    x = np.asarray(x, np.float32)
    weight = np.asarray(weight, np.float32)
    bias = np.asarray(bias, np.float32)
    comb = np.asarray(combine_array, np.float32)
    mask = np.asarray(dispatch_mask, np.float32)

    # xh: (B, E, 128, NTT, I); xh[b, e, p, tt, j] = x[b, tt*128+p, e, j]
    xh = np.ascontiguousarray(
        x.reshape(B, NTT, 128, E, I).transpose(0, 3, 2, 1, 4)).astype(BF16)
    # mh: (B, E, 128, NTT, C) in fp8 e3m4
    mh = np.ascontiguousarray(
        mask.reshape(B, NTT, 128, E, C).transpose(0, 3, 2, 1, 4)).astype(F8E3)
    # cbp: (B, E, NTCH, 128, NCT*512) fp8;
    # cbp[b, e, tch, pc, ct*512+tq] = comb[b, tch*512+tq, e, ct*128+pc]
    cbp = np.ascontiguousarray(
        comb.reshape(B, NTCH, 512, E, NCT, 128).transpose(0, 3, 1, 5, 4, 2)
    ).astype(F8E3).reshape(B, E, NTCH, 128, NCT * 512)
    # wfp: (2, 128, E, 2048); wfp[k, j, e, oq] =
    #     weight.reshape(E, O, I)[e, k*2048+oq, j] -- 16KB/partition bursts
    wfa = np.ascontiguousarray(
        weight.reshape(E, 2, 2048, I).transpose(1, 3, 0, 2)).astype(BF16)
    # S[b, t] = sum_{e,c} comb[b, t, e, c] -- bias*S added on host in f32
    s = comb.sum(axis=(2, 3))
    idm = np.eye(128, dtype=BF16)

    in_maps = []
    for k in range(NCORES):
        b, h = k // 2, k % 2
        es = slice(h * EL, (h + 1) * EL)
        in_maps.append({
            "xh": np.ascontiguousarray(xh[b, es]),
            "mh": np.ascontiguousarray(mh[b, es]),
            "cbp": np.ascontiguousarray(cbp[b, es]),
            "wfp": np.ascontiguousarray(wfa[:, :, es, :]).reshape(
                2, 128, EL * 2048),
            "ident": idm,
        })
    return in_maps, s, bias


def kernel(x, weight, bias, combine_array, dispatch_mask):
    from concourse import bass_utils

    if "nc" not in _CACHE:
        _CACHE["nc"] = _build()
    nc = _CACHE["nc"]

    in_maps, s, bias_f = _prep_inputs(
        x, weight, bias, combine_array, dispatch_mask)
    res = bass_utils.run_bass_kernel_spmd(
        nc, in_maps, core_ids=list(range(NCORES)))
    out = np.empty((B, T, O), np.float32)
    for b in range(B):
        out[b] = res.results[2 * b]["out"].astype(np.float32)
        out[b] += res.results[2 * b + 1]["out"].astype(np.float32)
    out += s[:, :, None] * bias_f[None, None, :]
    return out


# revision 24
# speedup vs baseline: 1.0768x; 1.0123x over previous
"""Trainium2 Bass kernel for nn_ExpertsChooseMaskedExpand (MoE routing).

Reference computes (per batch b):
    xd[e,c,j] = sum_t mask[t,e,c] * x[t,e,j]          (dispatch)
    y[e,c,o]  = sum_j xd[e,c,j] * w[e,o,j] + bias[o]  (expert GEMM)
    out[t,o]  = sum_{e,c} comb[t,e,c] * y[e,c,o]      (combine)

We use associativity to contract comb with xd first:
    z[t,e,j] = sum_c comb[t,e,c] * xd[e,c,j]
    out[t,o] = sum_{e,j} z[t,e,j] * w[e,o,j] + bias[o] * S[t],
    S[t] = sum_{e,c} comb[t,e,c]
which cuts FLOPs ~3.4x and never materializes y (B,E,C,O).

Sharding: 8 cores; core k handles batch b=k//2 and expert group
h=k%2 (experts h*4..h*4+4) over ALL 4096 tokens. Each core produces a
partial out (T, O) summed over its 4 experts only; the host adds the
two partials of each batch pair (plus bias*S).

Precision: x, mask and comb stream as fp8 e3m4 (4 mantissa bits).
mask/comb quantization noise is zero-mean and incoherent while the
signal rides a coherent mean-0.5 path through both contractions, so
their final-output contribution is ~1.2e-3; x contributes ~1.15e-2
(no attenuation). Total measured ~1.2e-2 vs the 2e-2 gate. w and z
stay bf16 (the z@w GEMM operands must), so the PE runs at the bf16
floor: ~216ns per 128x128x512 matmul, ~277us of matmul stream/core.

Schedule: single SBUF/PSUM pool pair (no mid-kernel pool barrier).
Head = dispatch per expert, paced by the fp8 mask stream which owns
the sync ring exclusively; xh upfront loads are tiny (fp8), and all
other loads (cb, wf, later xh) are emitted on the scalar engine AFTER
e0's psD-dependent casts so they cannot steal early mask bandwidth.
Head PE idle is filled by staging tch0 combine chains over experts
0..1 into bf16 partials (pout); the tail finishes those tiles with a
2-matmul (e2,e3) chain merged via a DVE add at drain (no PE inject).
Tail is PE-bound at the N=512 issue floor; out writes ride the sync
engine (idle in tail) and the final t-chunk drains in 1024-col pair
tiles split across the sync+gpsimd rings to kill the flush tail.
"""

import numpy as np
import ml_dtypes

BF16 = ml_dtypes.bfloat16
F8E3 = ml_dtypes.float8_e3m4

B, T, E, C = 4, 4096, 8, 512
I = 128            # per-expert input features
O = 4096           # out_features
NCORES = 8
EL = E // 2        # experts per core
NTT = T // 128     # 32 token tiles
NCT = C // 128     # 4 c-tiles
NTCH = T // 512    # 8 t-chunks (z / combine granularity)
NOT2 = O // 1024   # 4 o-slices of the weight DRAM layout

STAGE_TCHS = (0,)  # t-chunks staged (e0+e1) in the head
STAGE_OTS = 8      # how many 512-col o-slices of those chunks to stage

_CACHE = {}


def _build():
    import concourse.bass as bass
    import concourse.tile as tile
    import concourse.bacc as bacc
    import concourse.mybir as mybir

    f32 = mybir.dt.float32
    bf16 = mybir.dt.bfloat16
    f8 = mybir.dt.float8e3
    ts = bass.ts

    nc = bacc.Bacc(None, target_bir_lowering=False, debug=False)

    xh = nc.dram_tensor("xh", [EL, 128, NTT, I], f8, kind="ExternalInput")
    mh = nc.dram_tensor("mh", [EL, 128, NTT, C], f8, kind="ExternalInput")
    # comb packed per (expert, t-chunk): contiguous 2KB/partition bursts
    cbp = nc.dram_tensor("cbp", [EL, NTCH, 128, NCT * 512], f8,
                         kind="ExternalInput")
    wfp = nc.dram_tensor("wfp", [NOT2, 128, EL * 1024], bf16,
                         kind="ExternalInput")
    ident = nc.dram_tensor("ident", [128, 128], bf16, kind="ExternalInput")
    out_d = nc.dram_tensor("out", [T, O], bf16, kind="ExternalOutput")

    with tile.TileContext(nc) as tc:
        with (
            tc.tile_pool(name="persist", bufs=1) as persist,
            tc.tile_pool(name="work", bufs=1) as work,
            tc.tile_pool(name="psum", bufs=1, space="PSUM") as psum,
        ):
            wf_sb = persist.tile([128, EL, O], bf16, tag="wf")
            id_sb = persist.tile([128, 128], bf16, tag="ident")
            nc.gpsimd.dma_start(id_sb[:], ident[:])

            xd = {}    # e -> xd tile [128c, (ct j)] bf16
            zt = {}    # (e, tch) -> z^T tile [128j, 512t] bf16
            pout = {}  # (tch, tt, ot) -> staged (e0+e1) partial, bf16

            def cb_load(e, tch, eng):
                cb_t = work.tile([128, NCT * 512], f8, tag="cb",
                                 bufs=8, name=f"cb{e}_{tch}")
                eng.dma_start(cb_t[:], cbp[e, tch])
                return cb_t

            def stage_b(e, tch, cb_t):
                # z^T[e][tch][j, t] = sum_c xd[e][c, j] * comb^T[c, t]
                ps_b = psum.tile([128, 512], f32, tag="psB", bufs=2,
                                 name=f"psB{e}_{tch}")
                for ct in range(NCT):
                    nc.tensor.matmul(
                        ps_b[:],
                        xd[e][:, ts(ct, 128)],
                        cb_t[:, ts(ct, 512)],
                        start=(ct == 0),
                        stop=(ct == NCT - 1),
                    )
                z_sb = persist.tile([128, 512], bf16, tag="zt",
                                    bufs=12, name=f"zt{e}_{tch}")
                nc.vector.tensor_copy(z_sb[:], ps_b[:])
                zt[(e, tch)] = z_sb

            def ps512(name, head_phase=False):
                # [128,512] f32 psum tiles. In the tail, two alternating
                # tags (psC 3 + psD 2 banks) give an effective 5-deep
                # rotation; the psD tag doubles as the dispatch accumulator,
                # so while those chains are open (head) only psC is used.
                if head_phase:
                    return psum.tile([128, 512], f32, tag="psC", bufs=3,
                                     name=name)
                k = ps512.k = getattr(ps512, "k", 0) + 1
                if k % 2:
                    return psum.tile([128, 512], f32, tag="psC", bufs=3,
                                     name=name)
                return psum.tile([128, 512], f32, tag="psD", bufs=2,
                                 name=name)

            # staged combine chains (e0+e1), emitted interleaved into the
            # e2/e3 dispatch chunk slots so they fill mask-wait PE idle
            stage_list = [(tch, tt, ot)
                          for ot in range(STAGE_OTS)
                          for tch in STAGE_TCHS
                          for tt in range(4)]
            n_slots = 2 * (NTT // 8)

            def pout_chain(tch, tt, ot):
                ps_p = ps512(f"psP{tch}_{tt}_{ot}", head_phase=True)
                for pe in (0, 1):
                    nc.tensor.matmul(
                        ps_p[:],
                        zt[(pe, tch)][:, ts(tt, 128)],
                        wf_sb[:, pe, ts(ot, 512)],
                        start=(pe == 0), stop=(pe == 1),
                    )
                po = persist.tile([128, 512], bf16, tag="po",
                                  bufs=4 * len(STAGE_TCHS) * STAGE_OTS,
                                  name=f"po{tch}_{tt}_{ot}")
                nc.vector.tensor_copy(po[:], ps_p[:])
                pout[(tch, tt, ot)] = po

            # ---- Head: dispatch per expert, mask owns the sync ring ----
            NQ = NTT // 8
            xh_tiles = {}
            mh_tiles = {}
            for e in range(EL):
                for q in range(NQ):
                    mh_t = work.tile([128, 8, C], f8, tag="mha", bufs=6,
                                     name=f"mh{e}_{q}")
                    if e == 0:
                        # fine-grained first chunks: dispatch starts sooner
                        nc.sync.dma_start(mh_t[:, 0:4, :],
                                          mh[e, :, q * 8:q * 8 + 4, :])
                        nc.sync.dma_start(mh_t[:, 4:8, :],
                                          mh[e, :, q * 8 + 4:q * 8 + 8, :])
                    else:
                        nc.sync.dma_start(mh_t[:],
                                          mh[e, :, q * 8:q * 8 + 8, :])
                    mh_tiles[(e, q)] = mh_t
            # scalar ring upfront: only xh0 (split) + xh1's first half --
            # everything else is dep-gated so e0's mask owns the bandwidth
            for e in range(2):
                xh_t = work.tile([128, NTT, I], f8, tag="xh", bufs=3,
                                 name=f"xh{e}")
                if e == 0:
                    nc.scalar.dma_start(xh_t[:, 0:16, :], xh[e, :, 0:16, :])
                    nc.scalar.dma_start(xh_t[:, 16:32, :], xh[e, :, 16:32, :])
                else:
                    nc.scalar.dma_start(xh_t[:, 0:16, :], xh[e, :, 0:16, :])
                xh_tiles[e] = xh_t
            cb_head = {}

            for e in range(EL):
                # xd^T accumulator: [128j, 512c], one chain over all tt
                ps_d = psum.tile([128, C], f32, tag="psD", bufs=2,
                                 name=f"psDd{e}")
                for q in range(NQ):
                    mh_t = mh_tiles[(e, q)]
                    for i in range(8):
                        tt = q * 8 + i
                        nc.tensor.matmul(
                            ps_d[:],
                            xh_tiles[e][:, tt, :],
                            mh_t[:, i, :],
                            start=(tt == 0),
                            stop=(tt == NTT - 1),
                        )
                    if e >= 2:
                        # staged chains after this slot's dispatch MMs: they
                        # fill the PE idle while the next mask chunk arrives
                        # and can never head-of-line block dispatch
                        slot = (e - 2) * NQ + q
                        k0 = len(stage_list) * slot // n_slots
                        k1 = len(stage_list) * (slot + 1) // n_slots
                        for (tch, tt_, ot) in stage_list[k0:k1]:
                            pout_chain(tch, tt_, ot)
                xdT_sb = work.tile([128, C], bf16, tag="xdT", bufs=2,
                                   name=f"xdT{e}")
                xd_sb = persist.tile([128, C], bf16, tag="xd", bufs=EL,
                                     name=f"xd{e}")
                for ct in range(NCT):
                    # scalar engine: keeps the cast off the vector queue
                    nc.scalar.copy(xdT_sb[:, ts(ct, 128)],
                                   ps_d[:, ts(ct, 128)])
                    ps_t = psum.tile([128, 128], bf16, tag="psT", bufs=1,
                                     name=f"psT{e}_{ct}")
                    nc.tensor.transpose(ps_t[:], xdT_sb[:, ts(ct, 128)],
                                        id_sb[:])
                    nc.vector.tensor_copy(xd_sb[:, ts(ct, 128)], ps_t[:])
                if e == 0:
                    # dep-gated loads: these triggers sit on the scalar
                    # engine after e0's psD-dependent casts, so their
                    # transfers start only once dispatch is underway and
                    # never steal early bandwidth from the mask stream.
                    # Order = consumption order in the schedule.
                    nc.scalar.dma_start(xh_tiles[1][:, 16:32, :],
                                        xh[1, :, 16:32, :])
                    for tch in STAGE_TCHS:
                        cb_head[(0, tch)] = cb_load(0, tch, nc.scalar)
                        cb_head[(1, tch)] = cb_load(1, tch, nc.scalar)
                    nc.scalar.dma_start(wf_sb[:, :, ts(0, 1024)], wfp[0])
                    xh_t2 = work.tile([128, NTT, I], f8, tag="xh", bufs=3,
                                      name="xh2")
                    nc.scalar.dma_start(xh_t2[:], xh[2])
                    xh_tiles[2] = xh_t2
                    nc.scalar.dma_start(wf_sb[:, :, ts(1, 1024)], wfp[1])
                    for tch in STAGE_TCHS:
                        cb_head[(2, tch)] = cb_load(2, tch, nc.scalar)
                    nc.scalar.dma_start(wf_sb[:, :, ts(2, 1024)], wfp[2])
                    xh_t3 = work.tile([128, NTT, I], f8, tag="xh", bufs=3,
                                      name="xh3")
                    nc.scalar.dma_start(xh_t3[:], xh[3])
                    xh_tiles[3] = xh_t3
                    for tch in STAGE_TCHS:
                        cb_head[(3, tch)] = cb_load(3, tch, nc.scalar)
                    nc.scalar.dma_start(wf_sb[:, :, ts(3, 1024)], wfp[3])
                xd[e] = xd_sb
                for tch in STAGE_TCHS:
                    stage_b(e, tch, cb_head[(e, tch)])

            # ---- Combine (PE-bound), stage B interleaved per t-chunk ----
            for tcg in range(NTCH):
                staged = tcg in STAGE_TCHS
                final = tcg == NTCH - 1
                for tt in range(tcg * 4, tcg * 4 + 4):
                    m = tt % 4
                    if not final:
                        out_sb = work.tile([128, O], bf16, tag="out",
                                           bufs=3, name=f"out{tt}")
                    for ot in range(NOT2 * 2):
                        if final and ot % 2 == 0:
                            # final t-chunk: small pair tiles, written out
                            # immediately on two rings -- no 1MB flush tail
                            out_sb = work.tile([128, 1024], bf16,
                                               tag="out7", bufs=12,
                                               name=f"out7_{tt}_{ot}")
                        osl = out_sb[:, ts(ot % 2 if final else ot, 512)]
                        ps_c = ps512(f"psC{tt}_{ot}")
                        es = ((2, 3) if staged and ot < STAGE_OTS
                              else (0, 1, 2, 3))
                        for k, e in enumerate(es):
                            nc.tensor.matmul(
                                ps_c[:],
                                zt[(e, tcg)][:, ts(m, 128)],
                                wf_sb[:, e, ts(ot, 512)],
                                start=(k == 0),
                                stop=(k == len(es) - 1),
                            )
                        if staged and ot < STAGE_OTS:
                            # merge the staged (e0+e1) partial at drain
                            nc.vector.tensor_tensor(
                                osl, ps_c[:], pout[(tcg, m, ot)][:],
                                mybir.AluOpType.add)
                        elif ot % 2 == 0:
                            nc.vector.tensor_copy(osl, ps_c[:])
                        else:
                            nc.scalar.copy(osl, ps_c[:])
                        if final and ot % 2 == 1:
                            eng = nc.gpsimd if (ot // 2) % 2 else nc.sync
                            eng.dma_start(
                                out_d[ts(tt, 128),
                                      (ot - 1) * 512:(ot + 1) * 512],
                                out_sb[:])
                    if not final:
                        nc.sync.dma_start(out_d[ts(tt, 128), :], out_sb[:])
                        # one z chain per token tile keeps the psB / vector
                        # load smooth
                        e = tt % 4
                        stage_b(e, tcg + 1, cb_load(e, tcg + 1, nc.gpsimd))

    nc.compile()
    return nc


def _prep_inputs(x, weight, bias, combine_array, dispatch_mask):
    """Host-side cast + re-layout for contiguous device DMA."""
    x = np.asarray(x, np.float32)
    weight = np.asarray(weight, np.float32)
    bias = np.asarray(bias, np.float32)
    comb = np.asarray(combine_array, np.float32)
    mask = np.asarray(dispatch_mask, np.float32)

    # xh: (B, E, 128, NTT, I); xh[b, e, p, tt, j] = x[b, tt*128+p, e, j]
    xh = np.ascontiguousarray(
        x.reshape(B, NTT, 128, E, I).transpose(0, 3, 2, 1, 4)).astype(F8E3)
    # mh: (B, E, 128, NTT, C) in fp8 e3m4
    mh = np.ascontiguousarray(
        mask.reshape(B, NTT, 128, E, C).transpose(0, 3, 2, 1, 4)).astype(F8E3)
    # cbp: (B, E, NTCH, 128, NCT*512) fp8;
    # cbp[b, e, tch, pc, ct*512+tq] = comb[b, tch*512+tq, e, ct*128+pc]
    cbp = np.ascontiguousarray(
        comb.reshape(B, NTCH, 512, E, NCT, 128).transpose(0, 3, 1, 5, 4, 2)
    ).astype(F8E3).reshape(B, E, NTCH, 128, NCT * 512)
    # wfp: (NOT2, 128, E, 1024); wfp[k, j, e, oq] =
    #     weight.reshape(E, O, I)[e, k*1024+oq, j] -- 8KB/partition bursts
    wfa = np.ascontiguousarray(
        weight.reshape(E, NOT2, 1024, I).transpose(1, 3, 0, 2)).astype(BF16)
    # S[b, t] = sum_{e,c} comb[b, t, e, c] -- bias*S added on host in f32
    s = comb.sum(axis=(2, 3))
    idm = np.eye(128, dtype=BF16)

    in_maps = []
    for k in range(NCORES):
        b, h = k // 2, k % 2
        es = slice(h * EL, (h + 1) * EL)
        in_maps.append({
            "xh": np.ascontiguousarray(xh[b, es]),
            "mh": np.ascontiguousarray(mh[b, es]),
            "cbp": np.ascontiguousarray(cbp[b, es]),
            "wfp": np.ascontiguousarray(wfa[:, :, es, :]).reshape(
                NOT2, 128, EL * 1024),
            "ident": idm,
        })
    return in_maps, s, bias


def kernel(x, weight, bias, combine_array, dispatch_mask):
    from concourse import bass_utils

    if "nc" not in _CACHE:
        _CACHE["nc"] = _build()
    nc = _CACHE["nc"]

    in_maps, s, bias_f = _prep_inputs(
        x, weight, bias, combine_array, dispatch_mask)
    res = bass_utils.run_bass_kernel_spmd(
        nc, in_maps, core_ids=list(range(NCORES)))
    out = np.empty((B, T, O), np.float32)
    for b in range(B):
        out[b] = res.results[2 * b]["out"].astype(np.float32)
        out[b] += res.results[2 * b + 1]["out"].astype(np.float32)
    out += s[:, :, None] * bias_f[None, None, :]
    return out
